# revision 19
# baseline (speedup 1.0000x reference)
"""Morphological dilation (max-plus 3x3 depthwise conv) on 8 Trainium2 cores.

out[b,c,y,x] = max_{i,j in 3x3} ( x_pad[b,c,y+i,x+j] + se[c,i,j] ),
x: [16,64,256,256] f32, se: [64,3,3] f32, pad=1 with CVAL=-10000.

Sharding: pure data parallel. Core k takes batches {2k, 2k+1}; the 2*64
(batch,channel) pairs map onto the 128 SBUF partitions, so se[c,i,j] is a
per-partition scalar. Spatial dims live on the free axis.

v6 pipeline. Engine-work lower bound (f16): the 8 tensor-tensor max folds
can only run on DVE (2x mode, 34.1us/plane -> 273us fixed); the 9 adds
split between DVE tensor_scalar (4x, 17.1us) and ACT Identity+bias (1x,
54.6us). LP optimum puts 6 adds on ACT, 3 on DVE -> both engines ~345us
busy incl. per-op overhead; everything else is pipeline slack. vs the v4
baseline (394us, ~34us DVE idle from first-chunk wait, acc-WAR stalls at
bufs=2, and tail drain):
  - per-block input tiles (3 rotating slots, 2-row halo re-loaded per
    block) instead of one monolithic padded-image tile; frees ~90KB/par
    of SBUF,
  - acc pool bufs=3 (store WAR gate moves 3 blocks back), 4 ACT tmp slots,
  - blocks [8, 32*7, 16, 8]: small first block starts compute early, fat
    interior blocks amortize per-instruction fixed costs (~240ns/op on
    ACT), small tail blocks shorten the drain,
  - loads 1-2 ride otherwise-idle store queues 5/6 (q0 serial chain made
    block 1 stall); se upload on q7,
  - the last block runs entirely on DVE (TS+TT for aligned taps, fused
    scalar_tensor_tensor for j=1) so the drain never waits on ACT, and
    its store uses a HW queue instead of SWDGE,
  - no ACT->DVE row splits: measured engine busy is balanced without them.
CAUTION: tile allocation order is a layout lottery — adding one small tile
shifted SBUF placement and slowed every interior TT fold by 17% (5310ns vs
4390ns for 8192 elems). The tail block therefore reuses the tmp_d
allocation the regular path would have made. Measured: 366us (v4: 394us).

Sync-wait budgets are 1 per instruction for every compute/DMA encoding used
here, so cross-engine handoffs go through 1-element "gate" ops that carry the
single foreign-semaphore wait (the consumer then only needs its own-engine
wait): DVE memset gates before each TT that reads an ACT tmp, ACT 1-element
Identity gates for tmp-slot reuse and input waits. A post-pass splits any
remaining multi-wait instruction into single-wait same-engine drains.
"""

import os
import numpy as np

B, C, H, W = 16, 64, 256, 256
NCORES = 8
P = 128  # partitions = (B // NCORES) * C
CVAL = -10000.0
KH = KW = 3

_DTYPE = os.environ.get("DILATION_DTYPE", "f16")

_nc_cache = {}
LAST_RESULTS = None  # BassKernelResults of the most recent run (for profiling)

# instruction name -> forced HWDGE queue index (consulted by the patched
# TileClockTick._assign_tick during scheduling)
_FORCED_HW_QUEUE = {}
_ASSIGN_PATCHED = False

# taps: (i, j) with per-tap scalar index t = 3*i + j
_DVE_TAPS = [(0, 0), (1, 0), (2, 0)]  # j=0 -> 4B-aligned reads
_ACT_TAPS = [(0, 1), (1, 1), (2, 1), (0, 2), (1, 2), (2, 2)]


def _patch_queue_assignment():
    global _ASSIGN_PATCHED
    if _ASSIGN_PATCHED:
        return
    import concourse.tile_sem_assignment as tsa

    orig = tsa.TileClockTick._assign_tick

    def _assign_tick(self, inst):
        forced = _FORCED_HW_QUEUE.get(getattr(inst, "name", None))
        if forced is None:
            return orig(self, inst)
        save = self.next_hw_dma_idx
        self.next_hw_dma_idx = forced
        try:
            return orig(self, inst)
        finally:
            self.next_hw_dma_idx = save

    tsa.TileClockTick._assign_tick = _assign_tick
    _ASSIGN_PATCHED = True


def _split_excess_waits(nc, mybir, max_waits: int = 1):
    """Walrus's per-encoding sync-wait slots are scarce (1 for most ops used
    here). Hoist all but `max_waits` waits of any instruction onto freshly
    inserted same-engine Drain instructions placed right before it."""
    n = 0
    for bb in nc.main_func.blocks:
        insts = bb.instructions
        i = 0
        while i < len(insts):
            ins = insts[i]
            si = ins.sync_info
            if si is not None and len(si.on_wait) > max_waits:
                waits = list(si.on_wait)
                keep = waits[-max_waits:]
                spill = waits[:-max_waits]
                new_insts = []
                for w in spill:
                    d = mybir.InstDrain(name=f"wsplit-{n}", ins=[], outs=[])
                    n += 1
                    d.engine = ins.engine
                    d.sync_info = mybir.SyncInfo(on_wait=[w], on_update=[])
                    new_insts.append(d)
                ins.sync_info = mybir.SyncInfo(
                    on_wait=keep, on_update=list(si.on_update)
                )
                insts[i:i] = new_insts
                i += len(new_insts)
            i += 1
        bb.instructions = insts


def _build(dtype_tag: str, h: int = H, nxin: int = 3, nslots: int = 4,
           accbufs: int = 3, split_waits: bool = True):
    import concourse.bass as bass
    import concourse.mybir as mybir
    from concourse.tile import TileContext, add_dep_helper

    _patch_queue_assignment()
    _FORCED_HW_QUEUE.clear()

    assert dtype_tag == "f16", "v5 layout is fp16-only"
    dt = mybir.dt.float16
    f32 = mybir.dt.float32
    add = mybir.AluOpType.add
    vmax = mybir.AluOpType.max
    ident = mybir.ActivationFunctionType.Identity

    nc = bass.Bass(trn_type="TRN2", num_swdge_queues=4)
    x_d = nc.declare_dram_parameter("x", [P, h, W], dt, isOutput=False)
    se_d = nc.declare_dram_parameter("sep", [P, KH * KW], f32, isOutput=False)
    out_d = nc.declare_dram_parameter("out", [P, h, W], dt, isOutput=True)

    # all block sizes EVEN: the DVE 4x tensor_scalar mode needs even dims.
    # Small first block -> compute starts as soon as a ~10-row load lands;
    # small last blocks -> short final store/drain. Fat interior blocks
    # amortize the per-instruction fixed costs (ACT pays ~240ns/op).
    blocks = [8] + [32] * 7 + [16, 8]
    assert sum(blocks) == h and all(b % 2 == 0 for b in blocks)
    nblocks = len(blocks)
    maxrows = max(blocks)

    with TileContext(nc) as tc:
        with (
            tc.tile_pool(name="const", bufs=1) as cpool,
            tc.tile_pool(name="xp", bufs=1) as xpool,
            tc.tile_pool(name="accp", bufs=accbufs) as apool,
            tc.tile_pool(name="tmpp", bufs=1) as tpool,
        ):
            se_t = cpool.tile([P, KH * KW], f32, name="se_t")
            se_dma = nc.sync.dma_start(out=se_t[:], in_=se_d[:])
            _FORCED_HW_QUEUE[se_dma.ins.name] = 7  # off the load-0 queue

            # Rotating per-block input tiles: xin row t = padded input row
            # y0-1+t for the block using the slot, cols 1..W hold data, cols
            # 0 and W+1 stay CVAL forever.
            xins = [
                xpool.tile([P, maxrows + 2, W + 2], dt, name=f"xin{s}")
                for s in range(nxin)
            ]
            for s in range(nxin):
                nc.vector.memset(xins[s][:, :, 0:1], CVAL)
                nc.vector.memset(xins[s][:, :, W + 1 : W + 2], CVAL)

            # scratch tiles for gates (tiny 1-element targets)
            dve_scr = cpool.tile([P, 4 * nblocks], dt, name="dve_scr")
            act_scr = cpool.tile([P, 4], dt, name="act_scr")
            act_src = cpool.tile([P, 1], dt, name="act_src")
            nc.vector.memset(act_src[:], 0.0)

            # ping-pong tmp tiles for the ACT adds
            tmps = [
                tpool.tile([P, maxrows, W], dt, name=f"tmp{i}") for i in range(nslots)
            ]

            # per-block load bounds; block k loads input rows [lo, hi] into
            # xin rows starting at dr (xin row t = padded input row y0-1+t)
            starts = []
            y0 = 0
            for rows in blocks:
                starts.append(y0)
                y0 += rows

            def emit_load(blk):
                """Returns the list of DMAs covering block `blk`'s input.
                Loads 0 and 1 are latency-critical (blocks 0/1 stall on
                them), so each is split in half across two otherwise-idle
                queues; later loads have blocks of slack and share q0."""
                rows, y0 = blocks[blk], starts[blk]
                s = blk % nxin
                lo = max(y0 - 1, 0)
                hi = min(y0 + rows, h - 1)
                dr = lo - (y0 - 1)
                n = hi - lo + 1
                if blk == 0:
                    half = n // 2
                    parts = [(lo, half, dr, 0), (lo + half, n - half, dr + half, 4)]
                elif blk == 1:
                    half = n // 2
                    parts = [(lo, half, dr, 5), (lo + half, n - half, dr + half, 6)]
                else:
                    parts = [(lo, n, dr, 6 if blk == 2 else 0)]
                lds = []
                for plo, pn, pdr, q in parts:
                    ld = nc.sync.dma_start(
                        out=xins[s][:, pdr : pdr + pn, 1 : W + 1],
                        in_=x_d[:, plo : plo + pn, :],
                    )
                    _FORCED_HW_QUEUE[ld.ins.name] = q
                    lds.append(ld)
                return lds

            # preload the first nxin blocks (their slots have no prior reader;
            # emitted on SP before any store so SP order stays acyclic)
            load_dmas = [emit_load(k) for k in range(nxin)]

            out_dmas = []
            pad_memsets = {}
            y0 = 0
            tmp_idx = 0
            last_tap = [None] * nblocks  # last op reading block k's xin slot
            last_fold = [None] * nblocks  # f45 of block k (frees tmp slots)
            fold01 = [None] * nblocks
            for blk, rows in enumerate(blocks):
                s = blk % nxin
                xin = xins[s]

                # top/bottom CVAL pad rows for the edge blocks
                if blk == 0:
                    nc.vector.memset(xin[:, 0:1, :], CVAL)
                if blk == nblocks - 1:
                    pm = nc.vector.memset(xin[:, rows + 1 : rows + 2, :], CVAL)
                    if blk >= nxin and last_tap[blk - nxin] is not None:
                        add_dep_helper(
                            pm.ins, last_tap[blk - nxin].ins, reason="xin WAR pad"
                        )

                acc = apool.tile([P, rows, W], dt, name="acc")
                # DVE-side gates: gw absorbs the store whose acc slot this
                # block reuses, gx the input-load wait.
                if blk >= accbufs:
                    gw = nc.vector.memset(dve_scr[:, 4 * blk + 1 : 4 * blk + 2], 0.0)
                    add_dep_helper(gw.ins, out_dmas[blk - accbufs].ins, reason="acc WAR")
                gx = nc.vector.memset(dve_scr[:, 4 * blk : 4 * blk + 1], 0.0)
                for _ld in load_dmas[blk]:
                    add_dep_helper(gx.ins, _ld.ins, reason="input chunk")
                # ACT-side gate for the input chunk
                ga = nc.scalar.activation(
                    act_scr[:, 0:1], act_src[:, 0:1], ident, bias=se_t[:, 0:1]
                )
                for _ld in load_dmas[blk]:
                    add_dep_helper(ga.ins, _ld.ins, reason="input chunk/ACT")

                def act_add(tap, gate_tt):
                    """Emit one ACT add into the next tmp slot; gate_tt (if
                    set) is a fold whose completion frees the slot (and, by
                    the cumulative DVE semaphore, every earlier fold)."""
                    nonlocal tmp_idx
                    t_i, t_j = tap
                    ti = tmp_idx % nslots
                    tmp_idx += 1
                    sidx = 3 * t_i + t_j
                    if gate_tt is not None:
                        gt = nc.scalar.activation(
                            act_scr[:, 1:2], act_src[:, 0:1], ident,
                            bias=se_t[:, 0:1],
                        )
                        add_dep_helper(gt.ins, gate_tt.ins, reason="tmp WAR gate")
                    a = nc.scalar.activation(
                        tmps[ti][:, 0:rows, :],
                        xin[:, t_i : t_i + rows, t_j : t_j + W],
                        ident,
                        bias=se_t[:, sidx : sidx + 1],
                    )
                    return ti, a

                def act_fold_pair(p0, p1):
                    """Fold two finished ACT tmps; one gate on the later add
                    covers both (same-engine retirement is in-order)."""
                    gm = nc.vector.memset(dve_scr[:, 4 * blk + 2 : 4 * blk + 3], 0.0)
                    add_dep_helper(gm.ins, p1[1].ins, reason="ACT pair ready")
                    nc.vector.tensor_tensor(
                        acc[:], acc[:], tmps[p0[0]][:, 0:rows, :], vmax
                    )
                    return nc.vector.tensor_tensor(
                        acc[:], acc[:], tmps[p1[0]][:, 0:rows, :], vmax
                    )

                # DVE-only taps: aligned TS init + 2x (TS add -> tmp_d, TT max)
                (i0, j0), *dve_rest = _DVE_TAPS
                nc.vector.tensor_scalar(
                    acc[:],
                    xin[:, i0 : i0 + rows, j0 : j0 + W],
                    se_t[:, 3 * i0 + j0 : 3 * i0 + j0 + 1],
                    None,
                    add,
                )
                if blk == nblocks - 1:
                    # tail block runs entirely on DVE so the kernel's drain
                    # never waits on the (lagging) ACT pipeline: aligned taps
                    # via TS+TT (through tmp_d, same allocation the regular
                    # path would make), odd-offset j=1 taps via fused STT.
                    tmp_t = tpool.tile([P, rows, W], dt, name="tmp_d")
                    for t_i, t_j in _DVE_TAPS[1:] + _ACT_TAPS:
                        sidx = 3 * t_i + t_j
                        src = xin[:, t_i : t_i + rows, t_j : t_j + W]
                        if t_j == 1:
                            nc.vector.scalar_tensor_tensor(
                                acc[:], src, se_t[:, sidx : sidx + 1], acc[:],
                                add, vmax,
                            )
                        else:
                            nc.vector.tensor_scalar(
                                tmp_t[:], src, se_t[:, sidx : sidx + 1], None, add
                            )
                            nc.vector.tensor_tensor(acc[:], acc[:], tmp_t[:], vmax)
                    last_fold[blk] = None
                    last_tap[blk] = None
                    use_hw = True
                    od = nc.sync.dma_start(
                        out=out_d[:, y0 : y0 + rows, :], in_=acc[:]
                    )
                    _FORCED_HW_QUEUE[od.ins.name] = 1 + (blk % 7)
                    out_dmas.append(od)
                    y0 += rows
                    continue
                # exact-shape tile: a full-tile write engages the DVE 4x
                # tensor_scalar mode (a slice of a larger tile runs 2x)
                tmp_d = tpool.tile([P, rows, W], dt, name="tmp_d")
                for t_i, t_j in dve_rest:
                    sidx = 3 * t_i + t_j
                    nc.vector.tensor_scalar(
                        tmp_d[:],
                        xin[:, t_i : t_i + rows, t_j : t_j + W],
                        se_t[:, sidx : sidx + 1],
                        None,
                        add,
                    )
                    nc.vector.tensor_tensor(acc[:], acc[:], tmp_d[:], vmax)
                # paired folds: adds p0(s0) p1(s1) fold01, p2(s2) p3(s3)
                # fold23, p4(s0) p5(s1) fold45. Slot-reuse gates: p0 waits
                # the previous block's last fold (covers all its slots, DVE
                # retires in order); p4 waits this block's fold01.
                p0 = act_add(_ACT_TAPS[0], last_fold[blk - 1] if blk else None)
                p1 = act_add(_ACT_TAPS[1], None)
                f01 = act_fold_pair(p0, p1)
                fold01[blk] = f01
                p2 = act_add(_ACT_TAPS[2], None)
                p3 = act_add(_ACT_TAPS[3], None)
                f23 = act_fold_pair(p2, p3)
                p4 = act_add(_ACT_TAPS[4], f01)
                p5 = act_add(_ACT_TAPS[5], None)
                f45 = act_fold_pair(p4, p5)
                last_fold[blk] = f45
                last_tap[blk] = p5[1]

                # input-slot WAR: emit the load that reuses this slot now
                # (after this block's last xin readers exist, before this
                # block's store, keeping SP program order acyclic) and gate
                # it on the deepest readers.
                if blk + nxin < nblocks:
                    lds = emit_load(blk + nxin)
                    for _ld in lds:
                        add_dep_helper(_ld.ins, p5[1].ins, reason="xin WAR")
                        add_dep_helper(_ld.ins, f45.ins, reason="xin WAR dve")
                    load_dmas.append(lds)

                # stores: HW queues 1..7 for the first 7 blocks, SWDGE for
                # the middle, and HW queue 1 again for the final block (its
                # store is tail-critical; queue 1's first store is long done)
                use_hw = blk < 7 or blk == nblocks - 1
                oeng = nc.sync if use_hw else nc.gpsimd
                od = oeng.dma_start(out=out_d[:, y0 : y0 + rows, :], in_=acc[:])
                if use_hw:
                    _FORCED_HW_QUEUE[od.ins.name] = 1 + (blk % 7)
                out_dmas.append(od)
                y0 += rows

    if split_waits:
        _split_excess_waits(nc, mybir)
    return nc


def _get_nc():
    key = (_DTYPE,)
    if key not in _nc_cache:
        _nc_cache[key] = _build(_DTYPE)
    return _nc_cache[key]


def kernel(x: np.ndarray, se: np.ndarray) -> np.ndarray:
    global LAST_RESULTS
    from concourse.bass_utils import run_bass_kernel_spmd

    np_dt = np.float16 if _DTYPE == "f16" else np.float32
    x = np.asarray(x)
    se = np.asarray(se)
    xs = np.ascontiguousarray(x).reshape(NCORES, P, H, W).astype(np_dt)
    sep = np.ascontiguousarray(
        np.tile(np.asarray(se, np.float32).reshape(C, KH * KW), (P // C, 1))
    )

    nc = _get_nc()
    in_maps = [{"x": xs[k], "sep": sep} for k in range(NCORES)]
    trace = bool(os.environ.get("DILATION_TRACE"))
    kwargs = {}
    if trace:
        kwargs["trace"] = True
        tmpdir = os.environ.get("DILATION_TRACE_DIR")
        if tmpdir:
            kwargs["tmpdir"] = tmpdir
    res = run_bass_kernel_spmd(nc, in_maps, list(range(NCORES)), **kwargs)
    LAST_RESULTS = res
    out = np.stack([res.results[k]["out"] for k in range(NCORES)])
    return out.reshape(B, C, H, W).astype(np.float32)
